# revision 9
# baseline (speedup 1.0000x reference)
"""Trainium2 Bass kernel for nn_GNN_53145925321329 (GNN message passing).

Key algebraic fact: the reference computes a full [B, N_ENT, D] segment-sum,
but the output only reads segment `entity[0]`:

    out = u * tanh(agg[:, e0, :] @ W0)
    agg[:, e0, :] = sum_{edges e: rows[e]==e0} rel_w[:, values[e]] * entity_emb[cols[e]]
                  = rel_w @ T,   T[r, :] = sum_{matches with value r} entity_emb[cols[e]]

So the only O(E) work is scanning rows == e0 (memory-bound, sharded
edge-parallel across the 8 cores per the sharding hint); the ~16 surviving
edges feed a tiny dense tail.

Single fused NEFF (one launch, 8 cores, SPMD):
  1. Stream the core's 200704 row ids ([128, 1568] f32) in 7 chunks,
     overlapped DMA (SP/ACT alternating) + DVE scan.
  2. DVE scan per chunk: one fused scalar_tensor_tensor
     (rows == e0) * (16384 + in_chunk_pos), accumulated per partition.
     acc == 0 -> no match; acc in [16384, 16384+W) -> exactly one match at
     pos = acc - 16384 (exact in f32); acc >= 2*16384 -> multi-match.
  3. DVE decode: per-partition validity (exactly one match across all
     chunks) + global edge offset g = p*1568 + pos.
  4. GpSimd indirect DMA gather #1: (col, value) int32 pair per partition
     from the core's interleaved cv shard at offset g.
  5. GpSimd indirect DMA gather #2: entity_emb row [8] per partition at
     offset col.
  6. DVE one-hot over relations weighted by validity; PE matmul
     emb_rows^T @ onehot -> per-core T^T partial [8, 12].
  7. Outputs: per-core accs [128, 7] (shipped early) + tpart [8, 12].

Host: sums the 8 T^T partials (the "psum" of segment partials), corrects
the (rare, ~1%) partitions holding >1 match by rescanning just those
1568-element windows (exact for any multiplicity), then applies the
O(200)-FLOP tail  out = u * tanh((u @ relT @ T) @ W0).
"""

import os

import numpy as np

import concourse.bacc as bacc
import concourse.mybir as mybir
import concourse.tile as tile
from concourse import bass
from concourse import bass_utils

# Opt-in NTFF profiling (test.py sets this; harness path stays untraced).
TRACE = os.environ.get("BASS_KERNEL_TRACE", "0") == "1"
LAST_EXEC_NS = []  # [(label, exec_time_ns), ...] per launch when TRACE

# Problem shapes (hardcoded per contract)
E = 1_600_000
D = 8
B = 8
R = 12
N_ENT = 100_000
N_CORES = 8
P = 128
NCH = 7              # scan chunks (DMA/compute overlap)
W = 224              # elements per (partition, chunk); NCH*W = 1568
COLS = NCH * W       # 1568 elements per partition
PER_CORE = P * COLS  # 200_704
E_PAD = PER_CORE * N_CORES
ENC = 16384.0        # single-match encoder base (pos sums stay exact in f32)

_CACHE = {}
f32 = mybir.dt.float32
i32 = mybir.dt.int32


def build_fused():
    """One NEFF, 8 SPMD cores: scan + decode + gathers + per-core T^T."""
    nc = bacc.Bacc("TRN2", debug=False, target_bir_lowering=False,
                   num_devices=N_CORES)
    rows_in = nc.dram_tensor("rows", [P, COLS], f32, kind="ExternalInput").ap()
    ent_in = nc.dram_tensor("ent", [P, 1], f32, kind="ExternalInput").ap()
    cv_in = nc.dram_tensor("cv", [PER_CORE, 2], i32, kind="ExternalInput").ap()
    emb_in = nc.dram_tensor("emb", [N_ENT, D], f32, kind="ExternalInput").ap()
    accs_out = nc.dram_tensor("accs", [P, NCH], f32, kind="ExternalOutput").ap()
    tpart_out = nc.dram_tensor("tpart", [D, R], f32, kind="ExternalOutput").ap()

    with tile.TileContext(nc) as tc:
        with (
            tc.tile_pool(name="const", bufs=1) as cpool,
            tc.tile_pool(name="sbuf", bufs=NCH + 1) as pool,
            tc.tile_pool(name="scr", bufs=2) as spool,
            tc.tile_pool(name="psum", bufs=1, space="PSUM") as psum,
        ):
            # --- constants (POOL iotas + DVE casts), overlap the streaming ---
            iota_wi = cpool.tile([P, W], i32)
            nc.gpsimd.iota(iota_wi[:], pattern=[[1, W]], base=int(ENC),
                           channel_multiplier=0)
            iota_wf = cpool.tile([P, W], f32)
            nc.vector.tensor_copy(iota_wf[:], iota_wi[:])

            cb_i = cpool.tile([P, NCH], i32)
            nc.gpsimd.iota(cb_i[:], pattern=[[W, NCH]], base=0,
                           channel_multiplier=0)
            cb_f = cpool.tile([P, NCH], f32)
            nc.vector.tensor_copy(cb_f[:], cb_i[:])

            pb_i = cpool.tile([P, 1], i32)
            nc.gpsimd.iota(pb_i[:], pattern=[[0, 1]], base=0,
                           channel_multiplier=COLS)
            pb_f = cpool.tile([P, 1], f32)
            nc.vector.tensor_copy(pb_f[:], pb_i[:])

            iotar_i = cpool.tile([P, R], i32)
            nc.gpsimd.iota(iotar_i[:], pattern=[[1, R]], base=0,
                           channel_multiplier=0)
            iotar_f = cpool.tile([P, R], f32)
            nc.vector.tensor_copy(iotar_f[:], iotar_i[:])

            ent_t = cpool.tile([P, 1], f32)
            nc.scalar.dma_start(ent_t[:], ent_in[:])

            # --- phase 1: stream rows, fused compare*iota accumulate ---
            accs_t = cpool.tile([P, NCH], f32)
            for ch in range(NCH):
                rt = pool.tile([P, W], f32, tag="rows")
                eng = nc.sync if ch % 2 == 0 else nc.scalar
                eng.dma_start(rt[:], rows_in[:, ch * W:(ch + 1) * W])
                mt = spool.tile([P, W], f32, tag="mask")
                nc.vector.scalar_tensor_tensor(
                    out=mt[:], in0=rt[:], scalar=ent_t[:, :1], in1=iota_wf[:],
                    op0=mybir.AluOpType.is_equal, op1=mybir.AluOpType.mult,
                    accum_out=accs_t[:, ch:ch + 1])

            # accs ship out early (overlaps the decode/gather tail)
            nc.scalar.dma_start(accs_out[:], accs_t[:])

            # --- phase 2: decode single-match position per partition ---
            dec = cpool.tile([P, 5 * NCH], f32)
            nz = dec[:, 0 * NCH:1 * NCH]
            va = dec[:, 1 * NCH:2 * NCH]
            vv = dec[:, 2 * NCH:3 * NCH]
            t1 = dec[:, 3 * NCH:4 * NCH]
            t2 = dec[:, 4 * NCH:5 * NCH]
            red = cpool.tile([P, 6], f32)
            s_nz = red[:, 0:1]
            s_v = red[:, 1:2]
            pos_p = red[:, 2:3]
            valid0 = red[:, 3:4]
            valid = red[:, 4:5]
            g_f = red[:, 5:6]

            nc.vector.tensor_scalar(out=nz, in0=accs_t[:], scalar1=0.5,
                                    scalar2=0.0, op0=mybir.AluOpType.is_gt,
                                    op1=mybir.AluOpType.add, accum_out=s_nz)
            nc.vector.tensor_scalar(out=va, in0=accs_t[:], scalar1=ENC - 0.5,
                                    scalar2=None, op0=mybir.AluOpType.is_gt)
            nc.vector.scalar_tensor_tensor(
                out=vv, in0=accs_t[:], scalar=ENC + W - 0.5, in1=va,
                op0=mybir.AluOpType.is_lt, op1=mybir.AluOpType.mult,
                accum_out=s_v)
            nc.vector.tensor_tensor(out=t1, in0=accs_t[:], in1=cb_f[:],
                                    op=mybir.AluOpType.add)
            nc.vector.scalar_tensor_tensor(
                out=t2, in0=t1, scalar=ENC, in1=vv,
                op0=mybir.AluOpType.subtract, op1=mybir.AluOpType.mult,
                accum_out=pos_p)
            nc.vector.tensor_scalar(out=valid0, in0=s_nz, scalar1=1.0,
                                    scalar2=None, op0=mybir.AluOpType.is_equal)
            nc.vector.scalar_tensor_tensor(
                out=valid, in0=s_v, scalar=1.0, in1=valid0,
                op0=mybir.AluOpType.is_equal, op1=mybir.AluOpType.mult)
            # g = pos_p*valid + p*COLS  (valid==1 implies pos_p in [0, COLS))
            nc.vector.scalar_tensor_tensor(
                out=g_f, in0=pos_p, scalar=valid, in1=pb_f[:],
                op0=mybir.AluOpType.mult, op1=mybir.AluOpType.add)
            g_i = cpool.tile([P, 1], i32)
            nc.vector.tensor_copy(g_i[:], g_f)

            # --- phase 3: gather (col, value) then the emb row ---
            cv_sb = cpool.tile([P, 2], i32)
            nc.gpsimd.indirect_dma_start(
                out=cv_sb[:], out_offset=None, in_=cv_in[:, :],
                in_offset=bass.IndirectOffsetOnAxis(ap=g_i[:, :1], axis=0))
            val_f = cpool.tile([P, 1], f32)
            nc.vector.tensor_copy(val_f[:], cv_sb[:, 1:2])
            emb_sb = cpool.tile([P, D], f32)
            nc.gpsimd.indirect_dma_start(
                out=emb_sb[:], out_offset=None, in_=emb_in[:, :],
                in_offset=bass.IndirectOffsetOnAxis(ap=cv_sb[:, 0:1], axis=0))

            # --- phase 4: per-core T^T = emb_rows^T @ (onehot(value)*valid) ---
            oh = cpool.tile([P, R], f32)
            nc.vector.scalar_tensor_tensor(
                out=oh[:], in0=iotar_f[:], scalar=val_f[:, :1],
                in1=valid.to_broadcast([P, R]),
                op0=mybir.AluOpType.is_equal, op1=mybir.AluOpType.mult)
            tpsum = psum.tile([D, R], f32)
            nc.tensor.matmul(out=tpsum[:], lhsT=emb_sb[:], rhs=oh[:],
                             start=True, stop=True)
            tsb = cpool.tile([D, R], f32)
            nc.vector.tensor_copy(tsb[:], tpsum[:])
            nc.sync.dma_start(tpart_out[:], tsb[:])
    nc.compile()
    return nc


def _get(name, builder, *args):
    key = (name,) + args
    if key not in _CACHE:
        _CACHE[key] = builder(*args)
    return _CACHE[key]


def _host_decode(accs_i):
    """Replicate the device decode exactly (integer math). Returns
    (valid [P], pos [P]) for one core's accs [P, NCH] int array."""
    nz = accs_i > 0
    v = (accs_i >= int(ENC)) & (accs_i < int(ENC) + W)
    s_nz = nz.sum(axis=1)
    s_v = v.sum(axis=1)
    valid = (s_nz == 1) & (s_v == 1)
    cb = np.arange(NCH, dtype=np.int64) * W
    pos = ((accs_i - int(ENC) + cb[None, :]) * v).sum(axis=1)
    return valid, pos


def kernel(user, entity, values, indices, user_emb, relation_emb, entity_emb,
           weight_0) -> np.ndarray:
    user = np.asarray(user)
    entity = np.asarray(entity)
    values = np.asarray(values)
    indices = np.asarray(indices)
    user_emb = np.asarray(user_emb, dtype=np.float32)
    relation_emb = np.asarray(relation_emb, dtype=np.float32)
    entity_emb = np.asarray(entity_emb, dtype=np.float32)
    weight_0 = np.asarray(weight_0, dtype=np.float32)

    ent0 = int(entity[0])

    # ---- shard prep (layout only; no O(E) compute) ----
    rows_pad = np.full(E_PAD, -1, dtype=np.float32)
    rows_pad[:E] = indices[0]
    shards = rows_pad.reshape(N_CORES, P, COLS)
    cv = np.zeros((E_PAD, 2), dtype=np.int32)
    cv[:E, 0] = indices[1]
    cv[:E, 1] = values
    cv_shards = cv.reshape(N_CORES, PER_CORE, 2)
    ent_b = np.full((P, 1), float(ent0), dtype=np.float32)

    nc = _get("fused", build_fused)
    res = bass_utils.run_bass_kernel_spmd(
        nc,
        [{"rows": np.ascontiguousarray(shards[c]), "ent": ent_b,
          "cv": np.ascontiguousarray(cv_shards[c]), "emb": entity_emb}
         for c in range(N_CORES)],
        core_ids=list(range(N_CORES)),
        trace=TRACE,
    )
    if TRACE:
        LAST_EXEC_NS.append(("fused", res.exec_time_ns))

    # ---- host: psum the per-core T^T partials ----
    tT = np.zeros((D, R), dtype=np.float32)
    for c in range(N_CORES):
        tT += res.results[c]["tpart"]

    # ---- host: exact correction for multi-match partitions (rare) ----
    for c in range(N_CORES):
        accs_i = np.rint(res.results[c]["accs"]).astype(np.int64)
        valid, _pos = _host_decode(accs_i)
        bad = np.nonzero(~valid & (accs_i.sum(axis=1) > 0))[0]
        for p in bad:
            win = shards[c, p]
            for w in np.nonzero(win == ent0)[0]:
                g = c * PER_CORE + p * COLS + int(w)
                col, val = int(cv[g, 0]), int(cv[g, 1])
                tT[:, val] += entity_emb[col]

    # ---- host: O(200)-FLOP dense tail ----
    u = user_emb[user]                        # [B, D]
    rel_w = u @ relation_emb.T                # [B, R]
    agg = rel_w @ tT.T                        # [B, D]
    rep = np.tanh(agg @ weight_0)             # [B, D]
    return (u * rep).astype(np.float32)


# revision 14
# speedup vs baseline: 1.0063x; 1.0063x over previous
"""Trainium2 Bass kernel for nn_GNN_53145925321329 (GNN message passing).

Key algebraic fact: the reference computes a full [B, N_ENT, D] segment-sum,
but the output only reads segment `entity[0]`:

    out = u * tanh(agg[:, e0, :] @ W0)
    agg[:, e0, :] = sum_{edges e: rows[e]==e0} rel_w[:, values[e]] * entity_emb[cols[e]]
                  = rel_w @ T,   T[r, :] = sum_{matches with value r} entity_emb[cols[e]]

So the only O(E) work is scanning rows == e0 (memory-bound, sharded
edge-parallel across the 8 cores per the sharding hint); the ~16 surviving
edges feed a tiny dense tail.

Single fused NEFF (one launch, 8 cores, SPMD):
  1. Stream the core's 200704 row ids ([128, 1568] f32) in 7 chunks,
     overlapped DMA (SP/ACT alternating) + DVE scan.
  2. DVE scan per chunk: one fused scalar_tensor_tensor
     (rows == e0) * (16384 + in_chunk_pos), accumulated per partition.
     acc == 0 -> no match; acc in [16384, 16384+W) -> exactly one match at
     pos = acc - 16384 (exact in f32); acc >= 2*16384 -> multi-match.
  3. DVE decode: per-partition validity (exactly one match across all
     chunks) + global edge offset g = p*1568 + pos.
  4. GpSimd indirect DMA gather #1: (col, value) int32 pair per partition
     from the core's interleaved cv shard at offset g.
  5. GpSimd indirect DMA gather #2: entity_emb row [8] per partition at
     offset col.
  6. DVE one-hot over relations weighted by validity; PE matmul
     emb_rows^T @ onehot -> per-core T^T partial [8, 12].
  7. Outputs: per-core accs [128, 7] (shipped early) + tpart [8, 12].

Host: sums the 8 T^T partials (the "psum" of segment partials), corrects
the (rare, ~1%) partitions holding >1 match by rescanning just those
1568-element windows (exact for any multiplicity), then applies the
O(200)-FLOP tail  out = u * tanh((u @ relT @ T) @ W0).
"""

import os

import numpy as np

import concourse.bacc as bacc
import concourse.mybir as mybir
import concourse.tile as tile
from concourse import bass
from concourse import bass_utils

# Opt-in NTFF profiling (test.py sets this; harness path stays untraced).
TRACE = os.environ.get("BASS_KERNEL_TRACE", "0") == "1"
LAST_EXEC_NS = []  # [(label, exec_time_ns), ...] per launch when TRACE

# Problem shapes (hardcoded per contract)
E = 1_600_000
D = 8
B = 8
R = 12
N_ENT = 100_000
N_CORES = 8
P = 128
NCH = 7              # scan chunks (DMA/compute overlap)
W = 224              # elements per (partition, chunk); NCH*W = 1568
COLS = NCH * W       # 1568 elements per partition
PER_CORE = P * COLS  # 200_704
E_PAD = PER_CORE * N_CORES
ENC = 16384.0        # single-match encoder base (pos sums stay exact in f32)

_CACHE = {}
f32 = mybir.dt.float32
i32 = mybir.dt.int32


def build_fused():
    """One NEFF, 8 SPMD cores: scan + decode + gathers + per-core T^T."""
    nc = bacc.Bacc("TRN2", debug=False, target_bir_lowering=False,
                   num_devices=N_CORES)
    rows_in = nc.dram_tensor("rows", [P, COLS], f32, kind="ExternalInput").ap()
    ent_in = nc.dram_tensor("ent", [P, 1], f32, kind="ExternalInput").ap()
    cv_in = nc.dram_tensor("cv", [PER_CORE, 2], i32, kind="ExternalInput").ap()
    emb_in = nc.dram_tensor("emb", [N_ENT, D], f32, kind="ExternalInput").ap()
    accs_out = nc.dram_tensor("accs", [P, NCH], f32, kind="ExternalOutput").ap()
    tpart_out = nc.dram_tensor("tpart", [D, R], f32, kind="ExternalOutput").ap()

    with tile.TileContext(nc) as tc:
        with (
            tc.tile_pool(name="const", bufs=1) as cpool,
            tc.tile_pool(name="sbuf", bufs=NCH + 1) as pool,
            tc.tile_pool(name="scr", bufs=2) as spool,
            tc.tile_pool(name="psum", bufs=1, space="PSUM") as psum,
        ):
            # --- constants (POOL f32 iotas; exact, values < 2^24), overlap
            # the streaming. iota_wf holds ENC + global in-row position.
            iota_wf = cpool.tile([P, COLS], f32)
            nc.gpsimd.iota(iota_wf[:], pattern=[[1, COLS]], base=int(ENC),
                           channel_multiplier=0,
                           allow_small_or_imprecise_dtypes=True)

            pb_f = cpool.tile([P, 1], f32)
            nc.gpsimd.iota(pb_f[:], pattern=[[0, 1]], base=0,
                           channel_multiplier=COLS,
                           allow_small_or_imprecise_dtypes=True)

            iotar_f = cpool.tile([P, R], f32)
            nc.gpsimd.iota(iotar_f[:], pattern=[[1, R]], base=0,
                           channel_multiplier=0,
                           allow_small_or_imprecise_dtypes=True)

            ent_t = cpool.tile([P, 1], f32)
            nc.scalar.dma_start(ent_t[:], ent_in[:])

            # --- phase 1: stream rows, fused compare*iota accumulate ---
            accs_t = cpool.tile([P, NCH], f32)
            for ch in range(NCH):
                rt = pool.tile([P, W], f32, tag="rows")
                eng = nc.sync if ch % 2 == 0 else nc.scalar
                eng.dma_start(rt[:], rows_in[:, ch * W:(ch + 1) * W])
                mt = spool.tile([P, W], f32, tag="mask")
                nc.vector.scalar_tensor_tensor(
                    out=mt[:], in0=rt[:], scalar=ent_t[:, :1],
                    in1=iota_wf[:, ch * W:(ch + 1) * W],
                    op0=mybir.AluOpType.is_equal, op1=mybir.AluOpType.mult,
                    accum_out=accs_t[:, ch:ch + 1])

            # accs ship out early (overlaps the decode/gather tail)
            nc.scalar.dma_start(accs_out[:], accs_t[:])

            # --- phase 2: decode single-match position per partition ---
            dec = cpool.tile([P, 4 * NCH], f32)
            nz = dec[:, 0 * NCH:1 * NCH]
            va = dec[:, 1 * NCH:2 * NCH]
            vv = dec[:, 2 * NCH:3 * NCH]
            t2 = dec[:, 3 * NCH:4 * NCH]
            red = cpool.tile([P, 6], f32)
            s_nz = red[:, 0:1]
            s_v = red[:, 1:2]
            pos_p = red[:, 2:3]
            valid0 = red[:, 3:4]
            valid = red[:, 4:5]
            g_f = red[:, 5:6]

            nc.vector.tensor_scalar(out=nz, in0=accs_t[:], scalar1=0.5,
                                    scalar2=0.0, op0=mybir.AluOpType.is_gt,
                                    op1=mybir.AluOpType.add, accum_out=s_nz)
            nc.vector.tensor_scalar(out=va, in0=accs_t[:], scalar1=ENC - 0.5,
                                    scalar2=None, op0=mybir.AluOpType.is_gt)
            nc.vector.scalar_tensor_tensor(
                out=vv, in0=accs_t[:], scalar=2 * ENC - 0.5, in1=va,
                op0=mybir.AluOpType.is_lt, op1=mybir.AluOpType.mult,
                accum_out=s_v)
            nc.vector.scalar_tensor_tensor(
                out=t2, in0=accs_t[:], scalar=ENC, in1=vv,
                op0=mybir.AluOpType.subtract, op1=mybir.AluOpType.mult,
                accum_out=pos_p)
            nc.vector.tensor_scalar(out=valid0, in0=s_nz, scalar1=1.0,
                                    scalar2=None, op0=mybir.AluOpType.is_equal)
            nc.vector.scalar_tensor_tensor(
                out=valid, in0=s_v, scalar=1.0, in1=valid0,
                op0=mybir.AluOpType.is_equal, op1=mybir.AluOpType.mult)
            # g = pos_p*valid + p*COLS  (valid==1 implies pos_p in [0, COLS))
            nc.vector.scalar_tensor_tensor(
                out=g_f, in0=pos_p, scalar=valid, in1=pb_f[:],
                op0=mybir.AluOpType.mult, op1=mybir.AluOpType.add)
            g_i = cpool.tile([P, 1], i32)
            nc.vector.tensor_copy(g_i[:], g_f)

            # --- phase 3: gather (col, value) then the emb row ---
            cv_sb = cpool.tile([P, 2], i32)
            nc.gpsimd.indirect_dma_start(
                out=cv_sb[:], out_offset=None, in_=cv_in[:, :],
                in_offset=bass.IndirectOffsetOnAxis(ap=g_i[:, :1], axis=0))
            val_f = cpool.tile([P, 1], f32)
            nc.vector.tensor_copy(val_f[:], cv_sb[:, 1:2])
            emb_sb = cpool.tile([P, D], f32)
            nc.gpsimd.indirect_dma_start(
                out=emb_sb[:], out_offset=None, in_=emb_in[:, :],
                in_offset=bass.IndirectOffsetOnAxis(ap=cv_sb[:, 0:1], axis=0))

            # --- phase 4: per-core T^T = emb_rows^T @ (onehot(value)*valid) ---
            oh = cpool.tile([P, R], f32)
            nc.vector.scalar_tensor_tensor(
                out=oh[:], in0=iotar_f[:], scalar=val_f[:, :1],
                in1=valid.to_broadcast([P, R]),
                op0=mybir.AluOpType.is_equal, op1=mybir.AluOpType.mult)
            tpsum = psum.tile([D, R], f32)
            nc.tensor.matmul(out=tpsum[:], lhsT=emb_sb[:], rhs=oh[:],
                             start=True, stop=True)
            tsb = cpool.tile([D, R], f32)
            nc.vector.tensor_copy(tsb[:], tpsum[:])
            nc.sync.dma_start(tpart_out[:], tsb[:])
    nc.compile()
    return nc


def _get(name, builder, *args):
    key = (name,) + args
    if key not in _CACHE:
        _CACHE[key] = builder(*args)
    return _CACHE[key]


def _host_decode(accs_i):
    """Replicate the device decode exactly (integer math). Returns
    (valid [P], pos [P]) for one core's accs [P, NCH] int array.
    accs encode ENC + global in-row position for single matches."""
    nz = accs_i > 0
    v = (accs_i >= int(ENC)) & (accs_i < 2 * int(ENC))
    s_nz = nz.sum(axis=1)
    s_v = v.sum(axis=1)
    valid = (s_nz == 1) & (s_v == 1)
    pos = ((accs_i - int(ENC)) * v).sum(axis=1)
    return valid, pos


def kernel(user, entity, values, indices, user_emb, relation_emb, entity_emb,
           weight_0) -> np.ndarray:
    user = np.asarray(user)
    entity = np.asarray(entity)
    values = np.asarray(values)
    indices = np.asarray(indices)
    user_emb = np.asarray(user_emb, dtype=np.float32)
    relation_emb = np.asarray(relation_emb, dtype=np.float32)
    entity_emb = np.asarray(entity_emb, dtype=np.float32)
    weight_0 = np.asarray(weight_0, dtype=np.float32)

    ent0 = int(entity[0])

    # ---- shard prep (layout only; no O(E) compute) ----
    rows_pad = np.full(E_PAD, -1, dtype=np.float32)
    rows_pad[:E] = indices[0]
    shards = rows_pad.reshape(N_CORES, P, COLS)
    cv = np.zeros((E_PAD, 2), dtype=np.int32)
    cv[:E, 0] = indices[1]
    cv[:E, 1] = values
    cv_shards = cv.reshape(N_CORES, PER_CORE, 2)
    ent_b = np.full((P, 1), float(ent0), dtype=np.float32)

    nc = _get("fused", build_fused)
    res = bass_utils.run_bass_kernel_spmd(
        nc,
        [{"rows": np.ascontiguousarray(shards[c]), "ent": ent_b,
          "cv": np.ascontiguousarray(cv_shards[c]), "emb": entity_emb}
         for c in range(N_CORES)],
        core_ids=list(range(N_CORES)),
        trace=TRACE,
    )
    if TRACE:
        LAST_EXEC_NS.append(("fused", res.exec_time_ns))

    # ---- host: psum the per-core T^T partials ----
    tT = np.zeros((D, R), dtype=np.float32)
    for c in range(N_CORES):
        tT += res.results[c]["tpart"]

    # ---- host: exact correction for multi-match partitions (rare) ----
    for c in range(N_CORES):
        accs_i = np.rint(res.results[c]["accs"]).astype(np.int64)
        valid, _pos = _host_decode(accs_i)
        bad = np.nonzero(~valid & (accs_i.sum(axis=1) > 0))[0]
        for p in bad:
            win = shards[c, p]
            for w in np.nonzero(win == ent0)[0]:
                g = c * PER_CORE + p * COLS + int(w)
                col, val = int(cv[g, 0]), int(cv[g, 1])
                tT[:, val] += entity_emb[col]

    # ---- host: O(200)-FLOP dense tail ----
    u = user_emb[user]                        # [B, D]
    rel_w = u @ relation_emb.T                # [B, R]
    agg = rel_w @ tT.T                        # [B, D]
    rep = np.tanh(agg @ weight_0)             # [B, D]
    return (u * rep).astype(np.float32)


# revision 19
# speedup vs baseline: 1.4602x; 1.4511x over previous
"""Trainium2 Bass kernel for nn_GNN_53145925321329 (GNN message passing).

Key algebraic fact: the reference computes a full [B, N_ENT, D] segment-sum,
but the output only reads segment `entity[0]`:

    out = u * tanh(agg[:, e0, :] @ W0)
    agg[:, e0, :] = sum_{edges e: rows[e]==e0} rel_w[:, values[e]] * entity_emb[cols[e]]
                  = rel_w @ T,   T[r, :] = sum_{matches with value r} entity_emb[cols[e]]

So the only O(E) work is scanning rows == e0. That scan is memory-bound and
is sharded edge-parallel across the 8 cores per the sharding hint; the ~16
surviving edges feed an O(200)-FLOP dense tail.

Device NEFF (one launch, 8 SPMD cores):
  1. Stream the core's 200704 row ids ([128, 1568] f32) in 7 chunks,
     DMA issue alternating SP/ACT, overlapped with the DVE scan.
  2. DVE scan per chunk: ONE fused scalar_tensor_tensor per chunk
     (rows == e0) * (16384 + global_row_pos), accumulated per partition:
     acc == 0 -> no match in that (partition, chunk);
     acc in [16384, 32768) -> exactly one match at pos = acc - 16384
     (exact integer arithmetic in f32);
     acc >= 32768 -> multi-match (count folded into the high bits).
  3. Per-core accs [128, 7] is the only output (the per-core segment-sum
     partial in position-encoded form).

Host ("gather/unshard" + psum of partials): decodes accs with exact integer
math, picks up the matched (col, value) pairs, rescans the rare
multi-match 1568-element windows (exact for any multiplicity), gathers the
<=16 entity_emb rows, and applies  out = u * tanh(((u @ relT) @ T) @ W0).

KERNEL_FULL_DEVICE=1 selects the alternative fully-fused NEFF that also
performs the candidate (col,value) gather, the entity_emb row gather
(GpSimd indirect DMAs) and the one-hot PE matmul reduction to per-core
T^T partials on device (~10.9us vs ~7.2us modeled; same host correction
escape hatch for multi-match partitions).
"""

import os

import numpy as np

import concourse.bacc as bacc
import concourse.mybir as mybir
import concourse.tile as tile
from concourse import bass
from concourse import bass_utils

# Opt-in NTFF profiling (test.py sets this; harness path stays untraced).
TRACE = os.environ.get("BASS_KERNEL_TRACE", "0") == "1"
LAST_EXEC_NS = []  # [(label, exec_time_ns), ...] per launch when TRACE
FULL_DEVICE = os.environ.get("KERNEL_FULL_DEVICE", "0") == "1"

# Problem shapes (hardcoded per contract)
E = 1_600_000
D = 8
B = 8
R = 12
N_ENT = 100_000
N_CORES = 8
P = 128
CHUNKS = [240, 240, 240, 240, 240, 240, 128]  # scan chunk widths
NCH = len(CHUNKS)
COLS = sum(CHUNKS)   # 1568 elements per partition
PER_CORE = P * COLS  # 200_704
E_PAD = PER_CORE * N_CORES
ENC = 16384.0        # single-match encoder base (pos sums stay exact in f32)

_CACHE = {}
f32 = mybir.dt.float32
i32 = mybir.dt.int32


def _emit_scan(nc, tc, cpool, pool, spool, rows_in, ent_in, accs_out):
    """Shared scan front-end: stream rows, fused compare*iota accumulate.

    Returns the SBUF accs tile ([P, NCH] f32, position-encoded counts)."""
    # ENC + global in-row position, generated as f32 directly (values
    # < 2^15, exactly representable; Pool runs it under the DMA stream).
    iota_wf = cpool.tile([P, COLS], f32)
    nc.gpsimd.iota(iota_wf[:], pattern=[[1, COLS]], base=int(ENC),
                   channel_multiplier=0,
                   allow_small_or_imprecise_dtypes=True)

    ent_t = cpool.tile([P, 1], f32)
    nc.scalar.dma_start(ent_t[:], ent_in[:])

    # Uneven chunks: the small final chunk shortens the trailing DVE op
    # that sits on the critical path behind the last DMA semaphore.
    accs_t = cpool.tile([P, NCH], f32)
    off = 0
    for ch, cw in enumerate(CHUNKS):
        rt = pool.tile([P, cw], f32, tag=f"rows{ch}")
        eng = nc.sync if ch % 2 == 0 else nc.scalar
        eng.dma_start(rt[:], rows_in[:, off:off + cw])
        mt = spool.tile([P, cw], f32, tag=f"mask{ch % 2}")
        nc.vector.scalar_tensor_tensor(
            out=mt[:], in0=rt[:], scalar=ent_t[:, :1],
            in1=iota_wf[:, off:off + cw],
            op0=mybir.AluOpType.is_equal, op1=mybir.AluOpType.mult,
            accum_out=accs_t[:, ch:ch + 1])
        off += cw

    nc.scalar.dma_start(accs_out[:], accs_t[:])
    return accs_t


def build_scan():
    """Scan-only NEFF: per-core accs is the only output."""
    nc = bacc.Bacc("TRN2", debug=False, target_bir_lowering=False,
                   num_devices=N_CORES)
    rows_in = nc.dram_tensor("rows", [P, COLS], f32, kind="ExternalInput").ap()
    ent_in = nc.dram_tensor("ent", [P, 1], f32, kind="ExternalInput").ap()
    accs_out = nc.dram_tensor("accs", [P, NCH], f32, kind="ExternalOutput").ap()
    with tile.TileContext(nc) as tc:
        with (
            tc.tile_pool(name="const", bufs=1) as cpool,
            tc.tile_pool(name="sbuf", bufs=NCH + 1) as pool,
            tc.tile_pool(name="scr", bufs=2) as spool,
        ):
            _emit_scan(nc, tc, cpool, pool, spool, rows_in, ent_in, accs_out)
    nc.compile()
    return nc


def build_fused():
    """Fused NEFF: scan + decode + indirect gathers + per-core T^T."""
    nc = bacc.Bacc("TRN2", debug=False, target_bir_lowering=False,
                   num_devices=N_CORES)
    rows_in = nc.dram_tensor("rows", [P, COLS], f32, kind="ExternalInput").ap()
    ent_in = nc.dram_tensor("ent", [P, 1], f32, kind="ExternalInput").ap()
    cv_in = nc.dram_tensor("cv", [PER_CORE, 2], i32, kind="ExternalInput").ap()
    emb_in = nc.dram_tensor("emb", [N_ENT, D], f32, kind="ExternalInput").ap()
    accs_out = nc.dram_tensor("accs", [P, NCH], f32, kind="ExternalOutput").ap()
    tpart_out = nc.dram_tensor("tpart", [D, R], f32, kind="ExternalOutput").ap()

    with tile.TileContext(nc) as tc:
        with (
            tc.tile_pool(name="const", bufs=1) as cpool,
            tc.tile_pool(name="sbuf", bufs=NCH + 1) as pool,
            tc.tile_pool(name="scr", bufs=2) as spool,
            tc.tile_pool(name="psum", bufs=1, space="PSUM") as psum,
        ):
            pb_f = cpool.tile([P, 1], f32)
            nc.gpsimd.iota(pb_f[:], pattern=[[0, 1]], base=0,
                           channel_multiplier=COLS,
                           allow_small_or_imprecise_dtypes=True)
            iotar_f = cpool.tile([P, R], f32)
            nc.gpsimd.iota(iotar_f[:], pattern=[[1, R]], base=0,
                           channel_multiplier=0,
                           allow_small_or_imprecise_dtypes=True)

            accs_t = _emit_scan(nc, tc, cpool, pool, spool, rows_in, ent_in,
                                accs_out)

            # --- decode single-match position per partition ---
            dec = cpool.tile([P, 4 * NCH], f32)
            nz = dec[:, 0 * NCH:1 * NCH]
            va = dec[:, 1 * NCH:2 * NCH]
            vv = dec[:, 2 * NCH:3 * NCH]
            t2 = dec[:, 3 * NCH:4 * NCH]
            red = cpool.tile([P, 5], f32)
            s_nz = red[:, 0:1]
            s_v = red[:, 1:2]
            pos_p = red[:, 2:3]
            valid0 = red[:, 3:4]
            valid = red[:, 4:5]

            nc.vector.tensor_scalar(out=nz, in0=accs_t[:], scalar1=0.5,
                                    scalar2=0.0, op0=mybir.AluOpType.is_gt,
                                    op1=mybir.AluOpType.add, accum_out=s_nz)
            nc.vector.tensor_scalar(out=va, in0=accs_t[:], scalar1=ENC - 0.5,
                                    scalar2=None, op0=mybir.AluOpType.is_gt)
            nc.vector.scalar_tensor_tensor(
                out=vv, in0=accs_t[:], scalar=2 * ENC - 0.5, in1=va,
                op0=mybir.AluOpType.is_lt, op1=mybir.AluOpType.mult,
                accum_out=s_v)
            nc.vector.scalar_tensor_tensor(
                out=t2, in0=accs_t[:], scalar=ENC, in1=vv,
                op0=mybir.AluOpType.subtract, op1=mybir.AluOpType.mult,
                accum_out=pos_p)
            nc.vector.tensor_scalar(out=valid0, in0=s_nz, scalar1=1.0,
                                    scalar2=None, op0=mybir.AluOpType.is_equal)
            nc.vector.scalar_tensor_tensor(
                out=valid, in0=s_v, scalar=1.0, in1=valid0,
                op0=mybir.AluOpType.is_equal, op1=mybir.AluOpType.mult)
            # g = pos_p*valid + p*COLS  (valid==1 implies pos_p in [0, COLS));
            # int32 output tile makes the dtype conversion part of the op.
            g_i = cpool.tile([P, 1], i32)
            nc.vector.scalar_tensor_tensor(
                out=g_i[:], in0=pos_p, scalar=valid, in1=pb_f[:],
                op0=mybir.AluOpType.mult, op1=mybir.AluOpType.add)

            # --- gather (col, value) pair then the entity_emb row ---
            cv_sb = cpool.tile([P, 2], i32)
            nc.gpsimd.indirect_dma_start(
                out=cv_sb[:], out_offset=None, in_=cv_in[:, :],
                in_offset=bass.IndirectOffsetOnAxis(ap=g_i[:, :1], axis=0))
            val_f = cpool.tile([P, 1], f32)
            nc.vector.tensor_copy(val_f[:], cv_sb[:, 1:2])
            emb_sb = cpool.tile([P, D], f32)
            nc.gpsimd.indirect_dma_start(
                out=emb_sb[:], out_offset=None, in_=emb_in[:, :],
                in_offset=bass.IndirectOffsetOnAxis(ap=cv_sb[:, 0:1], axis=0))

            # --- per-core T^T = emb_rows^T @ (onehot(value)*valid) ---
            oh = cpool.tile([P, R], f32)
            nc.vector.scalar_tensor_tensor(
                out=oh[:], in0=iotar_f[:], scalar=val_f[:, :1],
                in1=valid.to_broadcast([P, R]),
                op0=mybir.AluOpType.is_equal, op1=mybir.AluOpType.mult)
            tpsum = psum.tile([D, R], f32)
            nc.tensor.matmul(out=tpsum[:], lhsT=emb_sb[:], rhs=oh[:],
                             start=True, stop=True)
            tsb = cpool.tile([D, R], f32)
            nc.vector.tensor_copy(tsb[:], tpsum[:])
            nc.sync.dma_start(tpart_out[:], tsb[:])
    nc.compile()
    return nc


def _get(name, builder, *args):
    key = (name,) + args
    if key not in _CACHE:
        _CACHE[key] = builder(*args)
    return _CACHE[key]


def _host_decode(accs_i):
    """Replicate the device decode exactly (integer math). Returns
    (valid [P], pos [P]) for one core's accs [P, NCH] int array.
    accs encode ENC + global in-row position for single matches."""
    nz = accs_i > 0
    v = (accs_i >= int(ENC)) & (accs_i < 2 * int(ENC))
    s_nz = nz.sum(axis=1)
    s_v = v.sum(axis=1)
    valid = (s_nz == 1) & (s_v == 1)
    pos = ((accs_i - int(ENC)) * v).sum(axis=1)
    return valid, pos


def _tail(user, user_emb, relation_emb, weight_0, tT):
    u = user_emb[user]                        # [B, D]
    rel_w = u @ relation_emb.T                # [B, R]
    rep = np.tanh((rel_w @ tT.T) @ weight_0)  # [B, D]
    return (u * rep).astype(np.float32)


def kernel(user, entity, values, indices, user_emb, relation_emb, entity_emb,
           weight_0) -> np.ndarray:
    user = np.asarray(user)
    entity = np.asarray(entity)
    values = np.asarray(values)
    indices = np.asarray(indices)
    user_emb = np.asarray(user_emb, dtype=np.float32)
    relation_emb = np.asarray(relation_emb, dtype=np.float32)
    entity_emb = np.asarray(entity_emb, dtype=np.float32)
    weight_0 = np.asarray(weight_0, dtype=np.float32)

    ent0 = int(entity[0])

    # ---- shard prep (layout only; no O(E) compute) ----
    rows_pad = np.full(E_PAD, -1, dtype=np.float32)
    rows_pad[:E] = indices[0]
    shards = rows_pad.reshape(N_CORES, P, COLS)
    ent_b = np.full((P, 1), float(ent0), dtype=np.float32)

    if FULL_DEVICE:
        cv = np.zeros((E_PAD, 2), dtype=np.int32)
        cv[:E, 0] = indices[1]
        cv[:E, 1] = values
        cv_shards = cv.reshape(N_CORES, PER_CORE, 2)
        nc = _get("fused", build_fused)
        in_maps = [{"rows": np.ascontiguousarray(shards[c]), "ent": ent_b,
                    "cv": np.ascontiguousarray(cv_shards[c]),
                    "emb": entity_emb} for c in range(N_CORES)]
    else:
        nc = _get("scan", build_scan)
        in_maps = [{"rows": np.ascontiguousarray(shards[c]), "ent": ent_b}
                   for c in range(N_CORES)]

    res = bass_utils.run_bass_kernel_spmd(
        nc, in_maps, core_ids=list(range(N_CORES)), trace=TRACE)
    if TRACE:
        LAST_EXEC_NS.append(("scan", res.exec_time_ns))

    # ---- host: combine per-core partials (the "psum"/unshard step) ----
    tT = np.zeros((D, R), dtype=np.float32)
    if FULL_DEVICE:
        for c in range(N_CORES):
            tT += res.results[c]["tpart"]

    for c in range(N_CORES):
        accs_i = np.rint(res.results[c]["accs"]).astype(np.int64)
        valid, pos = _host_decode(accs_i)
        if not FULL_DEVICE:
            # single-match partitions resolved by the device scan
            for p in np.nonzero(valid)[0]:
                g = c * PER_CORE + p * COLS + int(pos[p])
                tT[:, values[g]] += entity_emb[indices[1][g]]
        # multi-match partitions (rare): exact rescan of just that window
        bad = np.nonzero(~valid & (accs_i.sum(axis=1) > 0))[0]
        for p in bad:
            win = shards[c, p]
            for w in np.nonzero(win == ent0)[0]:
                g = c * PER_CORE + p * COLS + int(w)
                if g < E:
                    tT[:, values[g]] += entity_emb[indices[1][g]]

    return _tail(user, user_emb, relation_emb, weight_0, tT)
